# revision 10
# baseline (speedup 1.0000x reference)
"""GCN block (3 layers) on 8 trn2 NeuronCores, data-parallel over batch.

Math: each layer is X' = (adj + I) @ leaky_relu(X @ W).
Let A = adj + I. Using (A @ H) @ W == A @ (H @ W), fold each layer's weight
into the previous layer's output so every layer is one big matmul against A:

    H0 = lrelu(X0 W0)                 (tiny, on-chip)
    G0 = H0 W1 ; Z1 = A G0 ; H1 = lrelu(Z1)
    G1 = H1 W2 ; Z2 = A G1 ; H2 = lrelu(Z2)
    G2 = H2     ; X3 = A G2           (final output)

Per core: 8 samples x 16 features = 128 = partition width. Layouts:
    T-layout  [c=(b,d), m]   (128 partitions, N free)
    N-layout  [m, c]         (m partitions, 128 free)
Big matmul: out = lhsT.T @ rhs with lhsT = G (N-layout, stationary) and
rhs = A^T tiles (streamed from HBM) -> Z^T in T-layout. The 16x16 weights
are expanded to 128x128 block-diagonal so the tiny matmuls run all 8
samples at once:  G = (H^T)^T @ W_blk  via lhsT = H^T tile (T-layout).
A^T = adj.T + I is built on the host (layout prep), streamed 3x per core.
"""

import numpy as np

N_FULL = 4096
D = 16
B_FULL = 64
NCORES = 8
B_CORE = B_FULL // NCORES  # 8
C = B_CORE * D  # 128 partitions
P = 128
NEG_SLOPE = 0.2

_CACHE = {}


def _leaky(nc, dest, ps, pool, width):
    """dest = leaky_relu(ps): tmp = ps*slope (PSUM->SBUF), dest = max(ps, tmp).

    Split in two DVE ops because one instruction may read PSUM only once.
    """
    import concourse.mybir as mybir

    tmp = pool.tile([P, width], mybir.dt.float32, tag="lk")
    nc.vector.tensor_scalar_mul(tmp[:], ps[:], NEG_SLOPE)
    nc.vector.tensor_max(dest, ps[:], tmp[:])


def _build_nc(n, free, dt_big_name):
    """Build the Bass module (per-core program). Cached per config."""
    import concourse.bass as bass
    import concourse.mybir as mybir
    import concourse.tile as tile
    from concourse import bacc

    f32 = mybir.dt.float32
    dt_big = getattr(mybir.dt, dt_big_name)

    nt = n // P        # number of 128-row m-tiles
    nch = n // free    # output column chunks
    mb = n // free     # m panels of `free` rows (4 m-tiles each)
    tpb = free // P    # m-tiles per panel

    nc = bacc.Bacc(
        "TRN2", target_bir_lowering=False, debug=False, num_devices=NCORES
    )
    xt_h = nc.dram_tensor("xt", [C, n], f32, kind="ExternalInput")
    at_h = nc.dram_tensor("at", [n, n], dt_big, kind="ExternalInput")
    w_h = nc.dram_tensor("wt", [4, P, P], f32, kind="ExternalInput")
    out_h = nc.dram_tensor("out", [C, n], f32, kind="ExternalOutput")

    with tile.TileContext(nc) as tc:
        with (
            tc.tile_pool(name="const", bufs=1) as constp,
            tc.tile_pool(name="ht", bufs=2) as htp,
            tc.tile_pool(name="g", bufs=2) as gp,
            tc.tile_pool(name="ats", bufs=3) as atp,
            tc.tile_pool(name="outp", bufs=1) as outp,
            tc.tile_pool(name="lk", bufs=2) as lkp,
            tc.tile_pool(name="psb", bufs=2, space="PSUM") as psb,
            tc.tile_pool(name="pst", bufs=2, space="PSUM") as pst,
        ):
            xt_sb = constp.tile([C, n], f32)
            nc.sync.dma_start(xt_sb[:], xt_h[:])
            w_sb = constp.tile([P, 4, P], f32)
            nc.sync.dma_start(w_sb[:], w_h[:].rearrange("w p q -> p w q"))

            # H0^T = lrelu(W0_blk.T @ X0^T)  (T-layout)
            ht_cur = htp.tile([C, n], f32)
            for ch in range(nch):
                ps = psb.tile([P, free], f32)
                nc.tensor.matmul(
                    ps[:],
                    w_sb[:, 0, :],
                    xt_sb[:, ch * free:(ch + 1) * free],
                    start=True,
                    stop=True,
                )
                _leaky(nc, ht_cur[:, ch * free:(ch + 1) * free], ps, lkp, free)

            for layer in range(3):
                w_idx = layer + 1  # W1_blk, W2_blk, I128
                # tiny: G = (H^T)^T @ W_blk  (N-layout, dt_big)
                g_sb = gp.tile([P, n], dt_big)
                for mt in range(nt):
                    psg = pst.tile([P, P], f32)
                    nc.tensor.matmul(
                        psg[:],
                        ht_cur[:, mt * P:(mt + 1) * P],
                        w_sb[:, w_idx, :],
                        start=True,
                        stop=True,
                    )
                    nc.vector.tensor_copy(g_sb[:, mt * P:(mt + 1) * P], psg[:])

                # big: Z^T[:, nc_chunk] = sum_m G[m,:].T @ A^T[m, nc_chunk]
                last = layer == 2
                if last:
                    dest = outp.tile([C, n], f32)
                else:
                    dest = htp.tile([C, n], f32)
                for ncx in range(nch):
                    ps = psb.tile([P, free], f32)
                    for mbx in range(mb):
                        att = atp.tile([P, tpb, free], dt_big)
                        src = at_h[
                            mbx * free:(mbx + 1) * free,
                            ncx * free:(ncx + 1) * free,
                        ].rearrange("(t p) n -> p t n", p=P)
                        nc.sync.dma_start(att[:], src)
                        for t in range(tpb):
                            mt = mbx * tpb + t
                            nc.tensor.matmul(
                                ps[:],
                                g_sb[:, mt * P:(mt + 1) * P],
                                att[:, t, :],
                                start=(mt == 0),
                                stop=(mt == nt - 1),
                            )
                    dsl = dest[:, ncx * free:(ncx + 1) * free]
                    if last:
                        nc.vector.tensor_copy(dsl, ps[:])
                    else:
                        _leaky(nc, dsl, ps, lkp, free)
                ht_cur = dest

            nc.sync.dma_start(out_h[:], dest[:])

    nc.compile()
    return nc


def _get_nc(n, free, dt_big_name):
    key = (n, free, dt_big_name)
    if key not in _CACHE:
        _CACHE[key] = _build_nc(n, free, dt_big_name)
    return _CACHE[key]


def _block_diag(w, reps):
    """(D,D) -> (reps*D, reps*D) block diagonal, f32."""
    d = w.shape[0]
    out = np.zeros((reps * d, reps * d), dtype=np.float32)
    for b in range(reps):
        out[b * d:(b + 1) * d, b * d:(b + 1) * d] = w
    return out


def prepare_inputs(x, adj, Identity, W0, W1, W2, n=N_FULL, dt_big_name="float32"):
    """Host-side layout prep. Returns (in_maps, meta)."""
    b_full = x.shape[0]
    b_core = b_full // NCORES
    c = b_core * D

    if dt_big_name == "bfloat16":
        import ml_dtypes
        np_big = ml_dtypes.bfloat16
    else:
        np_big = np.float32

    at = np.ascontiguousarray(
        adj.T.astype(np.float32) + Identity.T.astype(np.float32)
    ).astype(np_big)

    reps = c // D
    w_all = np.stack(
        [
            _block_diag(np.asarray(W0, np.float32), reps),
            _block_diag(np.asarray(W1, np.float32), reps),
            _block_diag(np.asarray(W2, np.float32), reps),
            np.eye(c, dtype=np.float32),
        ]
    )

    # xt[core][b*D+d, m] = x[core*b_core + b, m, d]
    xf = np.asarray(x, np.float32)
    in_maps = []
    for core in range(NCORES):
        xs = xf[core * b_core:(core + 1) * b_core]      # (b_core, n, D)
        xt = np.ascontiguousarray(xs.transpose(0, 2, 1).reshape(c, n))
        in_maps.append({"xt": xt, "at": at, "wt": w_all})
    return in_maps


def gather_output(results, n=N_FULL, b_full=B_FULL):
    b_core = b_full // NCORES
    c = b_core * D
    out = np.empty((b_full, n, D), dtype=np.float32)
    for core in range(NCORES):
        oc = np.asarray(results[core]["out"], np.float32).reshape(b_core, D, n)
        out[core * b_core:(core + 1) * b_core] = oc.transpose(0, 2, 1)
    return out


def run(x, adj, Identity, W0, W1, W2, n=N_FULL, free=512,
        dt_big_name="float32", trace=False):
    from concourse.bass_utils import run_bass_kernel_spmd

    nc = _get_nc(n, free, dt_big_name)
    in_maps = prepare_inputs(x, adj, Identity, W0, W1, W2, n, dt_big_name)
    core_ids = list(range(NCORES))
    res = run_bass_kernel_spmd(nc, in_maps, core_ids, trace=trace)
    out = gather_output(res.results, n, x.shape[0])
    return out, res


def kernel(x, adj, Identity, W0, W1, W2):
    out, _ = run(x, adj, Identity, W0, W1, W2)
    return out


# revision 11
# speedup vs baseline: 1.6893x; 1.6893x over previous
"""GCN block (3 layers) on 8 trn2 NeuronCores, data-parallel over batch.

Math: each layer is X' = (adj + I) @ leaky_relu(X @ W).
Let A = adj + I. Using (A @ H) @ W == A @ (H @ W), fold each layer's weight
into the previous layer's output so every layer is one big matmul against A:

    H0 = lrelu(X0 W0)                 (tiny, on-chip)
    G0 = H0 W1 ; Z1 = A G0 ; H1 = lrelu(Z1)
    G1 = H1 W2 ; Z2 = A G1 ; H2 = lrelu(Z2)
    G2 = H2     ; X3 = A G2           (final output)

Per core: 8 samples x 16 features = 128 = partition width. Layouts:
    T-layout  [c=(b,d), m]   (128 partitions, N free)
    N-layout  [m, c]         (m partitions, 128 free)
Big matmul: out = lhsT.T @ rhs with lhsT = G (N-layout, stationary) and
rhs = A^T tiles (streamed from HBM) -> Z^T in T-layout. The 16x16 weights
are expanded to 128x128 block-diagonal so the tiny matmuls run all 8
samples at once:  G = (H^T)^T @ W_blk  via lhsT = H^T tile (T-layout).
A^T = adj.T + I is built on the host (layout prep), streamed 3x per core.
"""

import numpy as np

N_FULL = 4096
D = 16
B_FULL = 64
NCORES = 8
B_CORE = B_FULL // NCORES  # 8
C = B_CORE * D  # 128 partitions
P = 128
NEG_SLOPE = 0.2

_CACHE = {}


def _leaky(nc, dest, ps, pool, width):
    """dest = leaky_relu(ps): tmp = ps*slope (PSUM->SBUF), dest = max(ps, tmp).

    Split in two DVE ops because one instruction may read PSUM only once.
    """
    import concourse.mybir as mybir

    tmp = pool.tile([P, width], mybir.dt.float32, tag="lk")
    nc.vector.tensor_scalar_mul(tmp[:], ps[:], NEG_SLOPE)
    nc.vector.tensor_max(dest, ps[:], tmp[:])


def _build_nc(n, free, dt_big_name):
    """Build the Bass module (per-core program). Cached per config.

    dt_big_name: "float32" | "bfloat16" | "float32r".
      bfloat16: A^T/G/H^T/X^T/W stored bf16 (half DMA, full-rate PE).
      float32r: f32 storage, matmuls bitcast to fp32r (full-rate PE at
                free>=256, fp32 DMA cost, ~tf32 matmul precision).
    """
    import concourse.bass as bass
    import concourse.mybir as mybir
    import concourse.tile as tile
    from concourse import bacc

    f32 = mybir.dt.float32
    r32 = dt_big_name == "float32r"
    dt_st = f32 if r32 else getattr(mybir.dt, dt_big_name)  # storage dtype
    dt_act = dt_st  # activations/weights storage

    def mm(ap):
        # matmul-operand view: bitcast to fp32r in r32 mode
        return ap.bitcast(mybir.dt.float32r) if r32 else ap

    nt = n // P        # number of 128-row m-tiles
    nch = n // free    # output column chunks
    mb = n // free     # m panels of `free` rows (tpb m-tiles each)
    tpb = free // P    # m-tiles per panel

    nc = bacc.Bacc(
        "TRN2", target_bir_lowering=False, debug=False, num_devices=NCORES
    )
    xt_h = nc.dram_tensor("xt", [C, n], dt_act, kind="ExternalInput")
    at_h = nc.dram_tensor("at", [n, n], dt_st, kind="ExternalInput")
    w_h = nc.dram_tensor("wt", [4, P, P], dt_act, kind="ExternalInput")
    out_h = nc.dram_tensor("out", [C, n], f32, kind="ExternalOutput")

    with tile.TileContext(nc) as tc:
        with (
            tc.tile_pool(name="const", bufs=1) as constp,
            tc.tile_pool(name="ht", bufs=2) as htp,
            tc.tile_pool(name="g", bufs=2) as gp,
            tc.tile_pool(name="ats", bufs=3) as atp,
            tc.tile_pool(name="outp", bufs=1) as outp,
            tc.tile_pool(name="lk", bufs=2) as lkp,
            tc.tile_pool(name="psb", bufs=2, space="PSUM") as psb,
            tc.tile_pool(name="pst", bufs=2, space="PSUM") as pst,
        ):
            xt_sb = constp.tile([C, n], dt_act)
            nc.sync.dma_start(xt_sb[:], xt_h[:])
            w_sb = constp.tile([P, 4, P], dt_act)
            nc.sync.dma_start(w_sb[:], w_h[:].rearrange("w p q -> p w q"))

            # H0^T = lrelu(W0_blk.T @ X0^T)  (T-layout)
            ht_cur = htp.tile([C, n], dt_act)
            for ch in range(nch):
                ps = psb.tile([P, free], f32)
                nc.tensor.matmul(
                    ps[:],
                    mm(w_sb[:, 0, :]),
                    mm(xt_sb[:, ch * free:(ch + 1) * free]),
                    start=True,
                    stop=True,
                )
                _leaky(nc, ht_cur[:, ch * free:(ch + 1) * free], ps, lkp, free)

            for layer in range(3):
                w_idx = layer + 1  # W1_blk, W2_blk, I128
                # tiny: G = (H^T)^T @ W_blk  (N-layout)
                g_sb = gp.tile([P, n], dt_st)
                for mt in range(nt):
                    psg = pst.tile([P, P], f32)
                    nc.tensor.matmul(
                        psg[:],
                        mm(ht_cur[:, mt * P:(mt + 1) * P]),
                        mm(w_sb[:, w_idx, :]),
                        start=True,
                        stop=True,
                    )
                    nc.vector.tensor_copy(g_sb[:, mt * P:(mt + 1) * P], psg[:])

                # big: Z^T[:, nc_chunk] = sum_m G[m,:].T @ A^T[m, nc_chunk]
                last = layer == 2
                if last:
                    dest = outp.tile([C, n], f32)
                else:
                    dest = htp.tile([C, n], dt_act)
                for ncx in range(nch):
                    ps = psb.tile([P, free], f32)
                    for mbx in range(mb):
                        att = atp.tile([P, tpb, free], dt_st)
                        src = at_h[
                            mbx * free:(mbx + 1) * free,
                            ncx * free:(ncx + 1) * free,
                        ].rearrange("(t p) n -> p t n", p=P)
                        nc.sync.dma_start(att[:], src)
                        for t in range(tpb):
                            mt = mbx * tpb + t
                            nc.tensor.matmul(
                                ps[:],
                                mm(g_sb[:, mt * P:(mt + 1) * P]),
                                mm(att[:, t, :]),
                                start=(mt == 0),
                                stop=(mt == nt - 1),
                            )
                    dsl = dest[:, ncx * free:(ncx + 1) * free]
                    if last:
                        nc.vector.tensor_copy(dsl, ps[:])
                    else:
                        _leaky(nc, dsl, ps, lkp, free)
                ht_cur = dest

            nc.sync.dma_start(out_h[:], dest[:])

    nc.compile()
    return nc


def _get_nc(n, free, dt_big_name):
    key = (n, free, dt_big_name)
    if key not in _CACHE:
        _CACHE[key] = _build_nc(n, free, dt_big_name)
    return _CACHE[key]


def _block_diag(w, reps):
    """(D,D) -> (reps*D, reps*D) block diagonal, f32."""
    d = w.shape[0]
    out = np.zeros((reps * d, reps * d), dtype=np.float32)
    for b in range(reps):
        out[b * d:(b + 1) * d, b * d:(b + 1) * d] = w
    return out


def prepare_inputs(x, adj, Identity, W0, W1, W2, n=N_FULL, dt_big_name="float32"):
    """Host-side layout prep. Returns per-core input maps."""
    b_full = x.shape[0]
    b_core = b_full // NCORES
    c = b_core * D

    if dt_big_name == "bfloat16":
        import ml_dtypes
        np_st = ml_dtypes.bfloat16
    else:
        np_st = np.float32

    at = np.ascontiguousarray(
        adj.T.astype(np.float32) + Identity.T.astype(np.float32)
    ).astype(np_st)

    reps = c // D
    w_all = np.stack(
        [
            _block_diag(np.asarray(W0, np.float32), reps),
            _block_diag(np.asarray(W1, np.float32), reps),
            _block_diag(np.asarray(W2, np.float32), reps),
            np.eye(c, dtype=np.float32),
        ]
    ).astype(np_st)

    # xt[core][b*D+d, m] = x[core*b_core + b, m, d]
    xf = np.asarray(x, np.float32)
    in_maps = []
    for core in range(NCORES):
        xs = xf[core * b_core:(core + 1) * b_core]      # (b_core, n, D)
        xt = np.ascontiguousarray(xs.transpose(0, 2, 1).reshape(c, n)).astype(np_st)
        in_maps.append({"xt": xt, "at": at, "wt": w_all})
    return in_maps


def gather_output(results, n=N_FULL, b_full=B_FULL):
    b_core = b_full // NCORES
    c = b_core * D
    out = np.empty((b_full, n, D), dtype=np.float32)
    for core in range(NCORES):
        oc = np.asarray(results[core]["out"], np.float32).reshape(b_core, D, n)
        out[core * b_core:(core + 1) * b_core] = oc.transpose(0, 2, 1)
    return out


def run(x, adj, Identity, W0, W1, W2, n=N_FULL, free=512,
        dt_big_name="float32", trace=False):
    from concourse.bass_utils import run_bass_kernel_spmd

    nc = _get_nc(n, free, dt_big_name)
    in_maps = prepare_inputs(x, adj, Identity, W0, W1, W2, n, dt_big_name)
    core_ids = list(range(NCORES))
    res = run_bass_kernel_spmd(nc, in_maps, core_ids, trace=trace)
    out = gather_output(res.results, n, x.shape[0])
    return out, res


def kernel(x, adj, Identity, W0, W1, W2):
    out, _ = run(x, adj, Identity, W0, W1, W2)
    return out


# revision 13
# speedup vs baseline: 2.4515x; 1.4512x over previous
"""GCN block (3 layers) on 8 trn2 NeuronCores, data-parallel over batch.

Math: each layer is X' = (adj + I) @ leaky_relu(X @ W).
Let A = adj + I. Using (A @ H) @ W == A @ (H @ W), fold each layer's weight
into the previous layer's output so every layer is one big matmul against A:

    H0 = lrelu(X0 W0)                 (tiny, on-chip)
    G0 = H0 W1 ; Z1 = A G0 ; H1 = lrelu(Z1)
    G1 = H1 W2 ; Z2 = A G1 ; H2 = lrelu(Z2)
    G2 = H2     ; X3 = A G2           (final output)

Per core: 8 samples x 16 features = 128 = partition width. Layouts:
    T-layout  [c=(b,d), m]   (128 partitions, N free)
    N-layout  [m, c]         (m partitions, 128 free)
Big matmul: out = lhsT.T @ rhs with lhsT = G (N-layout, stationary) and
rhs = A^T tiles (streamed from HBM) -> Z^T in T-layout. The 16x16 weights
are expanded to 128x128 block-diagonal so the tiny matmuls run all 8
samples at once:  G = (H^T)^T @ W_blk  via lhsT = H^T tile (T-layout).
A^T = adj.T + I is built on the host (layout prep), streamed 3x per core.
"""

import numpy as np

N_FULL = 4096
D = 16
B_FULL = 64
NCORES = 8
B_CORE = B_FULL // NCORES  # 8
C = B_CORE * D  # 128 partitions
P = 128
NEG_SLOPE = 0.2

_CACHE = {}


def _leaky(nc, dest, ps, pool, width):
    """dest = leaky_relu(ps): tmp = ps*slope (PSUM->SBUF), dest = max(ps, tmp).

    Split in two DVE ops because one instruction may read PSUM only once.
    """
    import concourse.mybir as mybir

    tmp = pool.tile([P, width], mybir.dt.float32, tag="lk")
    nc.vector.tensor_scalar_mul(tmp[:], ps[:], NEG_SLOPE)
    nc.vector.tensor_max(dest, ps[:], tmp[:])


def _build_nc(n, free, dt_big_name):
    """Build the Bass module (per-core program). Cached per config.

    dt_big_name: "float32" | "bfloat16" | "float32r".
      bfloat16: A^T/G/H^T/X^T/W stored bf16 (half DMA, full-rate PE).
      float32r: f32 storage, matmuls bitcast to fp32r (full-rate PE at
                free>=256, fp32 DMA cost, ~tf32 matmul precision).
    """
    import concourse.bass as bass
    import concourse.mybir as mybir
    import concourse.tile as tile
    from concourse import bacc

    f32 = mybir.dt.float32
    r32 = dt_big_name == "float32r"
    dt_st = f32 if r32 else getattr(mybir.dt, dt_big_name)  # storage dtype
    dt_act = dt_st  # activations/weights storage

    def mm(ap):
        # matmul-operand view: bitcast to fp32r in r32 mode
        return ap.bitcast(mybir.dt.float32r) if r32 else ap

    nt = n // P        # number of 128-row m-tiles
    nch = n // free    # output column chunks
    mb = n // free     # m panels of `free` rows (tpb m-tiles each)
    tpb = free // P    # m-tiles per panel

    nc = bacc.Bacc(
        "TRN2", target_bir_lowering=False, debug=False, num_devices=NCORES
    )
    xt_h = nc.dram_tensor("xt", [C, n], dt_act, kind="ExternalInput")
    at_h = nc.dram_tensor("at", [n, n], dt_st, kind="ExternalInput")
    w_h = nc.dram_tensor("wt", [4, P, P], dt_act, kind="ExternalInput")
    out_h = nc.dram_tensor("out", [C, n], f32, kind="ExternalOutput")

    with tile.TileContext(nc) as tc:
        with (
            tc.tile_pool(name="const", bufs=1) as constp,
            tc.tile_pool(name="ht", bufs=2) as htp,
            tc.tile_pool(name="g", bufs=2) as gp,
            tc.tile_pool(name="ats", bufs=3) as atp,
            tc.tile_pool(name="outp", bufs=1) as outp,
            tc.tile_pool(name="lk", bufs=2) as lkp,
            tc.tile_pool(name="ps", bufs=8, space="PSUM") as psp,
        ):
            xt_sb = constp.tile([C, n], dt_act)
            nc.sync.dma_start(xt_sb[:], xt_h[:])
            w_sb = constp.tile([P, 4, P], dt_act)
            nc.sync.dma_start(w_sb[:], w_h[:].rearrange("w p q -> p w q"))

            # H0^T = lrelu(W0_blk.T @ X0^T)  (T-layout)
            ht_cur = htp.tile([C, n], dt_act)
            for ch in range(nch):
                ps = psp.tile([P, free], f32, tag="ps")
                nc.tensor.matmul(
                    ps[:],
                    mm(w_sb[:, 0, :]),
                    mm(xt_sb[:, ch * free:(ch + 1) * free]),
                    start=True,
                    stop=True,
                )
                _leaky(nc, ht_cur[:, ch * free:(ch + 1) * free], ps, lkp, free)

            for layer in range(3):
                w_idx = layer + 1  # W1_blk, W2_blk, I128
                # tiny: G = (H^T)^T @ W_blk  (N-layout)
                g_sb = gp.tile([P, n], dt_st)
                for mt in range(nt):
                    psg = psp.tile([P, P], f32, tag="ps")
                    nc.tensor.matmul(
                        psg[:],
                        mm(ht_cur[:, mt * P:(mt + 1) * P]),
                        mm(w_sb[:, w_idx, :]),
                        start=True,
                        stop=True,
                    )
                    nc.vector.tensor_copy(g_sb[:, mt * P:(mt + 1) * P], psg[:])

                # big: Z^T = sum_m G[m,:].T @ A^T[m, :]
                # m-outer: stream full row-panels of A^T (fat contiguous
                # DMA runs); all nch psum banks accumulate in parallel;
                # one LDWEIGHTS per (panel, t) serves nch matmuls.
                last = layer == 2
                if last:
                    dest = outp.tile([C, n], f32)
                else:
                    dest = htp.tile([C, n], dt_act)
                ps_list = [
                    psp.tile([P, free], f32, tag="ps", name=f"psc{i}")
                    for i in range(nch)
                ]
                for mbx in range(mb):
                    att = atp.tile([P, tpb, n], dt_st)
                    src = at_h[mbx * free:(mbx + 1) * free, :].rearrange(
                        "(t p) n -> p t n", p=P
                    )
                    eng = nc.sync if (mbx % 2 == 0) else nc.scalar
                    eng.dma_start(att[:], src)
                    for t in range(tpb):
                        mt = mbx * tpb + t
                        for ncx in range(nch):
                            nc.tensor.matmul(
                                ps_list[ncx][:],
                                mm(g_sb[:, mt * P:(mt + 1) * P]),
                                mm(att[:, t, ncx * free:(ncx + 1) * free]),
                                start=(mt == 0),
                                stop=(mt == nt - 1),
                            )
                for ncx in range(nch):
                    dsl = dest[:, ncx * free:(ncx + 1) * free]
                    if last:
                        nc.vector.tensor_copy(dsl, ps_list[ncx][:])
                    else:
                        _leaky(nc, dsl, ps_list[ncx], lkp, free)
                ht_cur = dest

            nc.sync.dma_start(out_h[:], dest[:])

    nc.compile()
    return nc


def _get_nc(n, free, dt_big_name):
    key = (n, free, dt_big_name)
    if key not in _CACHE:
        _CACHE[key] = _build_nc(n, free, dt_big_name)
    return _CACHE[key]


def _block_diag(w, reps):
    """(D,D) -> (reps*D, reps*D) block diagonal, f32."""
    d = w.shape[0]
    out = np.zeros((reps * d, reps * d), dtype=np.float32)
    for b in range(reps):
        out[b * d:(b + 1) * d, b * d:(b + 1) * d] = w
    return out


def prepare_inputs(x, adj, Identity, W0, W1, W2, n=N_FULL, dt_big_name="float32"):
    """Host-side layout prep. Returns per-core input maps."""
    b_full = x.shape[0]
    b_core = b_full // NCORES
    c = b_core * D

    if dt_big_name == "bfloat16":
        import ml_dtypes
        np_st = ml_dtypes.bfloat16
    else:
        np_st = np.float32

    at = np.ascontiguousarray(
        adj.T.astype(np.float32) + Identity.T.astype(np.float32)
    ).astype(np_st)

    reps = c // D
    w_all = np.stack(
        [
            _block_diag(np.asarray(W0, np.float32), reps),
            _block_diag(np.asarray(W1, np.float32), reps),
            _block_diag(np.asarray(W2, np.float32), reps),
            np.eye(c, dtype=np.float32),
        ]
    ).astype(np_st)

    # xt[core][b*D+d, m] = x[core*b_core + b, m, d]
    xf = np.asarray(x, np.float32)
    in_maps = []
    for core in range(NCORES):
        xs = xf[core * b_core:(core + 1) * b_core]      # (b_core, n, D)
        xt = np.ascontiguousarray(xs.transpose(0, 2, 1).reshape(c, n)).astype(np_st)
        in_maps.append({"xt": xt, "at": at, "wt": w_all})
    return in_maps


def gather_output(results, n=N_FULL, b_full=B_FULL):
    b_core = b_full // NCORES
    c = b_core * D
    out = np.empty((b_full, n, D), dtype=np.float32)
    for core in range(NCORES):
        oc = np.asarray(results[core]["out"], np.float32).reshape(b_core, D, n)
        out[core * b_core:(core + 1) * b_core] = oc.transpose(0, 2, 1)
    return out


def run(x, adj, Identity, W0, W1, W2, n=N_FULL, free=512,
        dt_big_name="float32", trace=False):
    from concourse.bass_utils import run_bass_kernel_spmd

    nc = _get_nc(n, free, dt_big_name)
    in_maps = prepare_inputs(x, adj, Identity, W0, W1, W2, n, dt_big_name)
    core_ids = list(range(NCORES))
    res = run_bass_kernel_spmd(nc, in_maps, core_ids, trace=trace)
    out = gather_output(res.results, n, x.shape[0])
    return out, res


def kernel(x, adj, Identity, W0, W1, W2):
    out, _ = run(x, adj, Identity, W0, W1, W2)
    return out
